# revision 6
# baseline (speedup 1.0000x reference)
"""LIF (leaky integrate-and-fire) spiking-neuron scan on 8 Trainium2 NeuronCores.

Reference semantics (per element, f32):
    h_t = v_{t-1} + (x_t - v_{t-1}) / 2        (tau = 2, v_reset = 0)
    s_t = (h_t >= 1)                           (spike, threshold v_th = 1)
    v_t = h_t * (1 - s_t)                      (hard reset)

Device formulation.  The /2 leak is absorbed by an exact power-of-two
rescaling done on the host: X_t = 2^t * x_t, state V_t = 2^{t+1} * v_t,
thresholds th_t = 2^{t+1}.  Then per step (f32, bit-identical rounding to
the unscaled recurrence since power-of-2 scaling commutes with fp
rounding):
    W_t = V_{t-1} + X_t                     (DVE tensor_tensor add)
    V_t = (W_t is_lt th_t) * W_t            (DVE scalar_tensor_tensor, fused
                                             compare+mask-multiply: hard reset)
Spike output on the otherwise-idle ACT engine:
    u_t = Sign(1.0 - W_t * 2^-(t+1))  in {+1: keep, -1: spike}  -> fp8e4m3
(host decodes spike := byte == 0xB8, i.e. fp8 -1.0).

DVE does 2 ops/step (down from the baseline's 3): the scalar_tensor_tensor
fuses the threshold compare and the reset multiply into one instruction.

Sharding: batch dim B=64 split across 8 cores (8 rows each); time stays
local (sequential scan).  DRAM layout is partition-major [128, T*512] so
every DMA segment is contiguous per partition.
"""

import os
import numpy as np

T, B, N = 64, 64, 8192
NCORES = 8
BL = B // NCORES          # batch rows per core
P = 128                   # SBUF partitions
F = (BL * N) // P         # free elems per partition per step  (512)

# timestep chunking for input DMA: small first chunks prime the pipeline
LOAD_CHUNKS = [1, 1, 2, 4] + [8] * 7
assert sum(LOAD_CHUNKS) == T
SG = 8                    # spike-store granularity (timesteps per output DMA)

_built = {}


def _build():
    if "nc" in _built:
        return _built["nc"]

    from contextlib import ExitStack
    import concourse.mybir as mybir
    from concourse import bacc, tile

    # Slim the kernel-exit choreography (same as the proven baseline): the
    # trailing all_engine_barrier after the sem clears only orders them
    # against later instructions, of which there are none at kernel end.
    from concourse.vector_clock import ScopedClock

    def _slim_drain_and_barrier(self, tick_clock, wait_clock):
        drain_inst = self.nc.sync.drain()
        wait_clock.add_sem_waits(
            drain_inst.ins, ScopedClock({None: tick_clock.global_clock})
        )
        self.nc.all_engine_barrier()
        popped = self.nc._tile_sem_poison_stack.pop()
        assert popped is self._sem_poison
        self.nc.clear_and_free_semaphores(list(self.sems.allocated().values()))

    tile.TileContext._drain_and_barrier = _slim_drain_and_barrier

    nc = bacc.Bacc("TRN2", target_bir_lowering=False, debug=False)
    # partition-major layouts: [P, T*F] so per-partition bytes are contiguous
    x_ext = nc.dram_tensor("x", [P, T * F], mybir.dt.float32, kind="ExternalInput")
    u_ext = nc.dram_tensor("u", [P, T * F], mybir.dt.float8e4, kind="ExternalOutput")

    add = mybir.AluOpType.add
    mult = mybir.AluOpType.mult
    is_lt = mybir.AluOpType.is_lt
    Sign = mybir.ActivationFunctionType.Sign

    with tile.TileContext(nc) as tc:
        with ExitStack() as ctx:
            xp = ctx.enter_context(tc.tile_pool(name="xp", bufs=1))
            wp = ctx.enter_context(tc.tile_pool(name="wp", bufs=4))
            up = ctx.enter_context(tc.tile_pool(name="up", bufs=2))
            vp = ctx.enter_context(tc.tile_pool(name="vp", bufs=1))

            # whole input resident in SBUF (128 KiB of the 208 KiB/partition)
            x_all = xp.tile([P, T * F], mybir.dt.float32)
            t0 = 0
            for i, ch in enumerate(LOAD_CHUNKS):
                # all loads on the SP ring: transfers aggregate over the 16
                # SDMA engines regardless of ring, and keeping ACT's
                # sequencer free of DMA work lets the signs run undisturbed
                nc.sync.dma_start(
                    out=x_all[:, t0 * F:(t0 + ch) * F],
                    in_=x_ext[:, t0 * F:(t0 + ch) * F],
                )
                t0 += ch

            v = vp.tile([P, F], mybir.dt.float32)

            ug = None
            for t in range(T):
                if t % SG == 0:
                    ug = up.tile([P, SG * F], mybir.dt.float8e4, tag="ug")

                if t == 0:
                    # V_{-1} = 0 so W_0 = X_0: read x directly, no add
                    ws = x_all[:, 0:F]
                else:
                    w = wp.tile([P, F], mybir.dt.float32, tag="w")
                    nc.vector.tensor_tensor(w[:], v[:], x_all[:, t * F:(t + 1) * F], add)
                    ws = w[:]

                # spike decision on ACT: u = sign(1 - W/th) in {+1 keep, -1 spike}
                nc.scalar.activation(
                    ug[:, (t % SG) * F:(t % SG + 1) * F],
                    ws,
                    Sign,
                    bias=1.0,
                    scale=-(2.0 ** -(t + 1)),
                )

                if t < T - 1:
                    # hard reset fused into one DVE op:
                    # V = (W < th) * W   with th = 2^{t+1}
                    nc.vector.scalar_tensor_tensor(
                        v[:], ws, float(2.0 ** (t + 1)), ws, is_lt, mult
                    )

                if t < T - SG:
                    if t % SG == SG - 1:
                        nc.gpsimd.dma_start(
                            out=u_ext[:, (t - SG + 1) * F:(t + 1) * F],
                            in_=ug[:],
                        )
                elif t in (T - SG + 3, T - SG + 5, T - SG + 6, T - 1):
                    # taper the final group's stores ([4,2,1,1] steps) so the
                    # kernel-exit drain only waits on a tiny last transfer
                    lo = {T - SG + 3: 0, T - SG + 5: 4, T - SG + 6: 6, T - 1: 7}[t]
                    eng = nc.gpsimd if t != T - 1 else nc.sync
                    eng.dma_start(
                        out=u_ext[:, ((T - SG) + lo) * F:(t + 1) * F],
                        in_=ug[:, lo * F:(t % SG + 1) * F],
                    )

    if int(os.environ.get("LIF_STRIP_DVE_WAITS", "1")):
        # The tile framework routes even same-engine dependencies through
        # semaphores; each DVE->DVE hop costs ~140ns of sem round-trip
        # (~10us over the 63-step scan).  The DVE executes in order, so
        # program order already guarantees its own RAW/WAR hazards: strip
        # waits on sems that only DVE instructions ever update, keeping all
        # cross-engine waits (DMA arrivals, ACT sign progress) intact.
        f = nc.m.functions[0]
        upd = {}
        dma_sems = set()
        for b in f.blocks:
            for i in b.instructions:
                si = i.sync_info
                if si:
                    for u in si.on_update:
                        if u.sync_type == "semaphore":
                            upd.setdefault(u.id, set()).add(i.engine)
                            if isinstance(i, mybir.InstDMA):
                                # DMA completion sems fire when the async
                                # transfer lands, not in engine order
                                dma_sems.add(u.id)
        for eng in (mybir.EngineType.DVE, mybir.EngineType.Activation):
            own_only = {
                sid
                for sid, engs in upd.items()
                if engs == {eng} and sid not in dma_sems
            }
            for b in f.blocks:
                keep_insts = []
                for i in b.instructions:
                    si = i.sync_info
                    if i.engine == eng and si and si.on_wait:
                        kept = [
                            w
                            for w in si.on_wait
                            if not (
                                w.sync_type == "semaphore" and w.id in own_only
                            )
                        ]
                        if len(kept) != len(si.on_wait):
                            si.on_wait = kept
                    # drop wait-only sem instructions whose condition was
                    # stripped: they are pure no-ops on the engine queue
                    if (
                        isinstance(i, mybir.InstEventSemaphore)
                        and i.engine == eng
                        and i.sync_info is not None
                        and not i.sync_info.on_wait
                        and not i.sync_info.on_update
                    ):
                        continue
                    keep_insts.append(i)
                if len(keep_insts) != len(b.instructions):
                    b.set_instructions(keep_insts)

    nc.compile()
    _built["nc"] = nc
    return nc


def _install_ntff_hook() -> bool:
    """Provide antenv.axon_hooks (absent in this image) so that
    run_bass_kernel_spmd(trace=True) can capture NTFF profiles via the
    ctypes hook that trn_agent_boot already implements."""
    try:
        from antenv.axon_hooks import get_axon_ntff_profile_hook  # noqa: F401
        return True
    except ImportError:
        pass
    try:
        import sys
        import types
        import antenv
        from trn_agent_boot.trn_boot import _ntff_profile_via_ctypes

        hook = _ntff_profile_via_ctypes("/opt/axon/libaxon_pjrt.so")
        if hook is None:
            return False
        mod = types.ModuleType("antenv.axon_hooks")
        state = {"hook": hook}
        mod.get_axon_ntff_profile_hook = lambda: state["hook"]
        mod.set_axon_ntff_profile_hook = lambda h: state.__setitem__("hook", h)
        sys.modules["antenv.axon_hooks"] = mod
        antenv.axon_hooks = mod
        return True
    except Exception:
        return False


def kernel(x: np.ndarray) -> np.ndarray:
    import concourse.bass_utils as bass_utils

    nc = _build()

    x = np.asarray(x)
    assert x.shape == (T, B, N) and x.dtype == np.float32

    # exact power-of-two prescale: X_t = 2^t * x_t (commutes with fp rounding)
    scales = np.exp2(np.arange(T, dtype=np.float32))
    xs = x * scales[:, None, None]

    in_maps = []
    for c in range(NCORES):
        # [T, BL*N] -> [T, P, F] -> [P, T, F] -> [P, T*F]  (partition-major)
        shard = (
            xs[:, c * BL:(c + 1) * BL, :]
            .reshape(T, P, F)
            .transpose(1, 0, 2)
            .reshape(P, T * F)
        )
        in_maps.append({"x": np.ascontiguousarray(shard)})

    trace = bool(int(os.environ.get("LIF_TRACE", "0")))
    if trace:
        trace = _install_ntff_hook()
        # artifact upload has no bucket in this container; neuter it
        bass_utils.upload_artifacts = lambda tmpdir: tmpdir

    try:
        res = bass_utils.run_bass_kernel_spmd(
            nc, in_maps, list(range(NCORES)), trace=trace
        )
    except Exception:
        if not trace:
            raise
        res = bass_utils.run_bass_kernel_spmd(
            nc, in_maps, list(range(NCORES)), trace=False
        )
    _built["last_result"] = res

    out = np.empty((T, B, N), np.float32)
    for c in range(NCORES):
        u = np.asarray(res.results[c]["u"])          # fp8e4m3 [P, T*F]
        bits = u.view(np.uint8).reshape(P, T, F).transpose(1, 0, 2)
        # spike <=> sign() returned -1.0 (0xB8 in fp8e4m3); sign()==0 (exact
        # threshold hit, measure-zero) decodes as no-spike
        spikes = (bits == 0xB8).astype(np.float32).reshape(T, BL, N)
        out[:, c * BL:(c + 1) * BL, :] = spikes
    return out


# revision 7
# speedup vs baseline: 1.0039x; 1.0039x over previous
"""LIF (leaky integrate-and-fire) spiking-neuron scan on 8 Trainium2 NeuronCores.

Reference semantics (per element, f32):
    h_t = v_{t-1} + (x_t - v_{t-1}) / 2        (tau = 2, v_reset = 0)
    s_t = (h_t >= 1)                           (spike, threshold v_th = 1)
    v_t = h_t * (1 - s_t)                      (hard reset)

Device formulation.  The /2 leak is absorbed by an exact power-of-two
rescaling done on the host: X_t = 2^t * x_t, state V_t = 2^{t+1} * v_t,
thresholds th_t = 2^{t+1}.  Then per step (f32, bit-identical rounding to
the unscaled recurrence since power-of-2 scaling commutes with fp
rounding):
    W_t = V_{t-1} + X_t                     (DVE tensor_tensor add)
    V_t = (W_t is_lt th_t) * W_t            (DVE scalar_tensor_tensor, fused
                                             compare+mask-multiply: hard reset)
Spike output on the otherwise-idle ACT engine:
    u_t = Sign(1.0 - W_t * 2^-(t+1))  in {+1: keep, -1: spike}  -> fp8e4m3
(host decodes spike := byte == 0xB8, i.e. fp8 -1.0).

DVE does 2 ops/step (down from the baseline's 3): the scalar_tensor_tensor
fuses the threshold compare and the reset multiply into one instruction.

Sharding: batch dim B=64 split across 8 cores (8 rows each); time stays
local (sequential scan).  DRAM layout is partition-major [128, T*512] so
every DMA segment is contiguous per partition.
"""

import os
import numpy as np

T, B, N = 64, 64, 8192
NCORES = 8
BL = B // NCORES          # batch rows per core
P = 128                   # SBUF partitions
F = (BL * N) // P         # free elems per partition per step  (512)

# timestep chunking for input DMA: small first chunks prime the pipeline
LOAD_CHUNKS = [1, 1, 2, 4] + [8] * 7
assert sum(LOAD_CHUNKS) == T
SG = 8                    # spike-store granularity (timesteps per output DMA)

_built = {}


def _build():
    if "nc" in _built:
        return _built["nc"]

    from contextlib import ExitStack
    import concourse.mybir as mybir
    from concourse import bacc, tile

    # Slim the kernel-exit choreography (same as the proven baseline): the
    # trailing all_engine_barrier after the sem clears only orders them
    # against later instructions, of which there are none at kernel end.
    from concourse.vector_clock import ScopedClock

    def _slim_drain_and_barrier(self, tick_clock, wait_clock):
        drain_inst = self.nc.sync.drain()
        wait_clock.add_sem_waits(
            drain_inst.ins, ScopedClock({None: tick_clock.global_clock})
        )
        self.nc.all_engine_barrier()
        popped = self.nc._tile_sem_poison_stack.pop()
        assert popped is self._sem_poison
        self.nc.clear_and_free_semaphores(list(self.sems.allocated().values()))

    tile.TileContext._drain_and_barrier = _slim_drain_and_barrier

    nc = bacc.Bacc("TRN2", target_bir_lowering=False, debug=False)
    # partition-major layouts: [P, T*F] so per-partition bytes are contiguous
    x_ext = nc.dram_tensor("x", [P, T * F], mybir.dt.float32, kind="ExternalInput")
    u_ext = nc.dram_tensor("u", [P, T * F], mybir.dt.float8e4, kind="ExternalOutput")

    add = mybir.AluOpType.add
    mult = mybir.AluOpType.mult
    is_lt = mybir.AluOpType.is_lt
    Sign = mybir.ActivationFunctionType.Sign

    with tile.TileContext(nc) as tc:
        with ExitStack() as ctx:
            xp = ctx.enter_context(tc.tile_pool(name="xp", bufs=1))
            wp = ctx.enter_context(tc.tile_pool(name="wp", bufs=4))
            up = ctx.enter_context(tc.tile_pool(name="up", bufs=2))
            vp = ctx.enter_context(tc.tile_pool(name="vp", bufs=1))

            # whole input resident in SBUF (128 KiB of the 208 KiB/partition)
            x_all = xp.tile([P, T * F], mybir.dt.float32)
            t0 = 0
            for i, ch in enumerate(LOAD_CHUNKS):
                # loads on the SP ring (transfers aggregate over the 16 SDMA
                # engines regardless of ring); the second priming chunk goes
                # out on ACT's ring so steps 0 and 1 land concurrently
                eng = nc.scalar if i == 1 else nc.sync
                eng.dma_start(
                    out=x_all[:, t0 * F:(t0 + ch) * F],
                    in_=x_ext[:, t0 * F:(t0 + ch) * F],
                )
                t0 += ch

            v = vp.tile([P, F], mybir.dt.float32)

            ug = None
            for t in range(T):
                if t % SG == 0:
                    ug = up.tile([P, SG * F], mybir.dt.float8e4, tag="ug")

                if t == 0:
                    # V_{-1} = 0 so W_0 = X_0: read x directly, no add
                    ws = x_all[:, 0:F]
                else:
                    w = wp.tile([P, F], mybir.dt.float32, tag="w")
                    nc.vector.tensor_tensor(w[:], v[:], x_all[:, t * F:(t + 1) * F], add)
                    ws = w[:]

                # spike decision on ACT: u = sign(1 - W/th) in {+1 keep, -1 spike}
                nc.scalar.activation(
                    ug[:, (t % SG) * F:(t % SG + 1) * F],
                    ws,
                    Sign,
                    bias=1.0,
                    scale=-(2.0 ** -(t + 1)),
                )

                if t < T - 1:
                    # hard reset fused into one DVE op:
                    # V = (W < th) * W   with th = 2^{t+1}
                    nc.vector.scalar_tensor_tensor(
                        v[:], ws, float(2.0 ** (t + 1)), ws, is_lt, mult
                    )

                if t < T - SG:
                    if t % SG == SG - 1:
                        nc.gpsimd.dma_start(
                            out=u_ext[:, (t - SG + 1) * F:(t + 1) * F],
                            in_=ug[:],
                        )
                elif t in (T - SG + 3, T - SG + 5, T - SG + 6, T - 1):
                    # taper the final group's stores ([4,2,1,1] steps) so the
                    # kernel-exit drain only waits on a tiny last transfer
                    lo = {T - SG + 3: 0, T - SG + 5: 4, T - SG + 6: 6, T - 1: 7}[t]
                    eng = nc.gpsimd if t != T - 1 else nc.sync
                    eng.dma_start(
                        out=u_ext[:, ((T - SG) + lo) * F:(t + 1) * F],
                        in_=ug[:, lo * F:(t % SG + 1) * F],
                    )

    if int(os.environ.get("LIF_STRIP_DVE_WAITS", "1")):
        # The tile framework routes even same-engine dependencies through
        # semaphores; each DVE->DVE hop costs ~140ns of sem round-trip
        # (~10us over the 63-step scan).  The DVE executes in order, so
        # program order already guarantees its own RAW/WAR hazards: strip
        # waits on sems that only DVE instructions ever update, keeping all
        # cross-engine waits (DMA arrivals, ACT sign progress) intact.
        f = nc.m.functions[0]
        upd = {}
        dma_sems = set()
        for b in f.blocks:
            for i in b.instructions:
                si = i.sync_info
                if si:
                    for u in si.on_update:
                        if u.sync_type == "semaphore":
                            upd.setdefault(u.id, set()).add(i.engine)
                            if isinstance(i, mybir.InstDMA):
                                # DMA completion sems fire when the async
                                # transfer lands, not in engine order
                                dma_sems.add(u.id)
        for eng in (mybir.EngineType.DVE, mybir.EngineType.Activation):
            own_only = {
                sid
                for sid, engs in upd.items()
                if engs == {eng} and sid not in dma_sems
            }
            for b in f.blocks:
                keep_insts = []
                for i in b.instructions:
                    si = i.sync_info
                    if i.engine == eng and si and si.on_wait:
                        kept = [
                            w
                            for w in si.on_wait
                            if not (
                                w.sync_type == "semaphore" and w.id in own_only
                            )
                        ]
                        if len(kept) != len(si.on_wait):
                            si.on_wait = kept
                    # drop wait-only sem instructions whose condition was
                    # stripped: they are pure no-ops on the engine queue
                    if (
                        isinstance(i, mybir.InstEventSemaphore)
                        and i.engine == eng
                        and i.sync_info is not None
                        and not i.sync_info.on_wait
                        and not i.sync_info.on_update
                    ):
                        continue
                    keep_insts.append(i)
                if len(keep_insts) != len(b.instructions):
                    b.set_instructions(keep_insts)

    nc.compile()
    _built["nc"] = nc
    return nc


def _install_ntff_hook() -> bool:
    """Provide antenv.axon_hooks (absent in this image) so that
    run_bass_kernel_spmd(trace=True) can capture NTFF profiles via the
    ctypes hook that trn_agent_boot already implements."""
    try:
        from antenv.axon_hooks import get_axon_ntff_profile_hook  # noqa: F401
        return True
    except ImportError:
        pass
    try:
        import sys
        import types
        import antenv
        from trn_agent_boot.trn_boot import _ntff_profile_via_ctypes

        hook = _ntff_profile_via_ctypes("/opt/axon/libaxon_pjrt.so")
        if hook is None:
            return False
        mod = types.ModuleType("antenv.axon_hooks")
        state = {"hook": hook}
        mod.get_axon_ntff_profile_hook = lambda: state["hook"]
        mod.set_axon_ntff_profile_hook = lambda h: state.__setitem__("hook", h)
        sys.modules["antenv.axon_hooks"] = mod
        antenv.axon_hooks = mod
        return True
    except Exception:
        return False


def kernel(x: np.ndarray) -> np.ndarray:
    import concourse.bass_utils as bass_utils

    nc = _build()

    x = np.asarray(x)
    assert x.shape == (T, B, N) and x.dtype == np.float32

    # exact power-of-two prescale: X_t = 2^t * x_t (commutes with fp rounding)
    scales = np.exp2(np.arange(T, dtype=np.float32))
    xs = x * scales[:, None, None]

    in_maps = []
    for c in range(NCORES):
        # [T, BL*N] -> [T, P, F] -> [P, T, F] -> [P, T*F]  (partition-major)
        shard = (
            xs[:, c * BL:(c + 1) * BL, :]
            .reshape(T, P, F)
            .transpose(1, 0, 2)
            .reshape(P, T * F)
        )
        in_maps.append({"x": np.ascontiguousarray(shard)})

    trace = bool(int(os.environ.get("LIF_TRACE", "0")))
    if trace:
        trace = _install_ntff_hook()
        # artifact upload has no bucket in this container; neuter it
        bass_utils.upload_artifacts = lambda tmpdir: tmpdir

    try:
        res = bass_utils.run_bass_kernel_spmd(
            nc, in_maps, list(range(NCORES)), trace=trace
        )
    except Exception:
        if not trace:
            raise
        res = bass_utils.run_bass_kernel_spmd(
            nc, in_maps, list(range(NCORES)), trace=False
        )
    _built["last_result"] = res

    out = np.empty((T, B, N), np.float32)
    for c in range(NCORES):
        u = np.asarray(res.results[c]["u"])          # fp8e4m3 [P, T*F]
        bits = u.view(np.uint8).reshape(P, T, F).transpose(1, 0, 2)
        # spike <=> sign() returned -1.0 (0xB8 in fp8e4m3); sign()==0 (exact
        # threshold hit, measure-zero) decodes as no-spike
        spikes = (bits == 0xB8).astype(np.float32).reshape(T, BL, N)
        out[:, c * BL:(c + 1) * BL, :] = spikes
    return out
